# revision 7
# baseline (speedup 1.0000x reference)
"""MiniBatchDiscrimination Trainium2 kernel.

Math (per reference):
    act = (x @ W).reshape(B, K, D)              # B=256, K=100, D=50
    l1[i,k,j] = sum_d |act[i,k,d] - act[j,k,d]|
    features[i,k] = sum_j exp(-l1[i,k,j])
    out = concat([x, features], axis=1)

Sharding: kernels K are independent in the pairwise stage, so we shard K
(columns of W) across the 8 cores -- 13 kernels (650 columns) per core,
K padded 100->104 with zero weight columns.  No collectives needed; each
core computes the full BxB pairwise work for its 13 kernels.

Per-core algorithm (transposed layout, act_T[col, j] with col=(k,d)):
  Phase A: act_T = W_shard.T @ x.T via PE (lhsT=W chunks, rhs=x.T), cast
    to bf16 in SBUF (and an fp32 upcast OF THE BF16 VALUES, used as the
    per-partition scalar so that the i==j diagonal is exactly 0).
  Phase B: for each batch row i:
    - DVE tensor_scalar: diff = abs_max(act_T - act_T[:, i], 0)  (bf16, 4x)
    - PE matmul with block-diagonal 0/1 stationary S reduces over d:
      l1[k, j] accumulated exactly in fp32 PSUM.  Four batch rows stack
      at 32-partition offsets of one PSUM bank via col-tiling
      (tile_position), two more pack along the free dim -> 8 rows/bank.
    - ScalarE: exp(-l1) with accum_out giving sum_j for free.
  Host reassembles features and concatenates with x (exact fp32).
"""

import numpy as np
import ml_dtypes
from contextlib import ExitStack

import concourse.bass as bass
import concourse.bacc as bacc
import concourse.tile as tile
from concourse import mybir
from concourse.bass_utils import run_bass_kernel_spmd

B = 256          # batch
IN_D = 1024      # input dim
NK = 13          # kernels per core (8*13 = 104 >= 100)
DK = 50          # dim per kernel
COLS = NK * DK   # 650 act_T rows per core
N_CORES = 8
# partition chunks of the 650 act_T rows
CHUNKS = [(0, 128), (128, 128), (256, 128), (384, 128), (512, 128), (640, 10)]
NCH = len(CHUNKS)

F32 = mybir.dt.float32
BF16 = mybir.dt.bfloat16


def build_nc():
    nc = bacc.Bacc()
    xT_d = nc.declare_dram_parameter("xT", [IN_D, B], BF16, isOutput=False)
    w_d = nc.declare_dram_parameter("w", [IN_D, COLS], BF16, isOutput=False)
    s_d = nc.declare_dram_parameter("s", [128, 416], BF16, isOutput=False)
    feat_d = nc.declare_dram_parameter("feat", [128, 64], F32, isOutput=True)
    a_d = nc.declare_dram_parameter("aout", [13, B], F32, isOutput=True)

    with ExitStack() as ctx:
        tc = ctx.enter_context(tile.TileContext(nc))
        const_pool = ctx.enter_context(tc.tile_pool(name="const", bufs=1))
        psum_a = ctx.enter_context(tc.tile_pool(name="psum_a", bufs=2, space="PSUM"))
        psum_b = ctx.enter_context(tc.tile_pool(name="psum_b", bufs=4, space="PSUM"))
        diff_pool = ctx.enter_context(tc.tile_pool(name="diff", bufs=4))
        junk_pool = ctx.enter_context(tc.tile_pool(name="junk", bufs=4))

        # ---- load inputs ----
        xt_tiles = []
        w_tiles = []
        for k in range(8):
            t = const_pool.tile([128, B], BF16, tag=f"xt{k}")
            nc.sync.dma_start(out=t[:], in_=xT_d[128 * k:128 * (k + 1), :])
            xt_tiles.append(t)
            tw = const_pool.tile([128, COLS], BF16, tag=f"w{k}")
            nc.sync.dma_start(out=tw[:], in_=w_d[128 * k:128 * (k + 1), :])
            w_tiles.append(tw)
        s_tile = const_pool.tile([128, 416], BF16, tag="s")
        nc.sync.dma_start(out=s_tile[:], in_=s_d[:])

        # ---- Phase A: act_T = W.T @ xT  (per 128-row chunk of act_T) ----
        act_bf = []   # bf16 streaming operand
        act_f32 = []  # fp32 upcast of the bf16 values (tensor_scalar scalar src)
        for t, (mstart, msz) in enumerate(CHUNKS):
            pa = psum_a.tile([msz, B], F32)
            for k in range(8):
                nc.tensor.matmul(
                    pa[:],
                    w_tiles[k][:, mstart:mstart + msz],
                    xt_tiles[k][:],
                    start=(k == 0),
                    stop=(k == 7),
                )
            tb = const_pool.tile([msz, B], BF16, tag=f"actb{t}")
            nc.vector.tensor_copy(tb[:], pa[:])
            tf = const_pool.tile([msz, B], F32, tag=f"actf{t}")
            nc.scalar.copy(tf[:], tb[:])
            act_bf.append(tb)
            act_f32.append(tf)
            if t == 0:
                tn = const_pool.tile([msz, B], F32, tag="actn0")
                nc.scalar.mul(tn[:], tb[:], -1.0)
                act_neg0 = tn

        # A[r, j] = sum_d act[(r,d), j]  (exact fp32 accumulation of bf16)
        pA = psum_a.tile([32, B], F32)
        for t, (mstart, msz) in enumerate(CHUNKS):
            nc.tensor.matmul(
                pA[:],
                s_tile[0:msz, 192 + 32 * t:192 + 32 * t + 32],
                act_bf[t][:],
                start=(t == 0),
                stop=(t == NCH - 1),
            )
        a_bf = const_pool.tile([13, B], BF16, tag="a_bf")
        nc.vector.tensor_copy(a_bf[:], pA[0:13, :])
        a_f32 = const_pool.tile([13, B], F32, tag="a_f32")
        nc.scalar.copy(a_f32[:], a_bf[:])
        nc.sync.dma_start(out=a_d[:], in_=a_f32[:])

        feat_tile = const_pool.tile([128, 64], F32, tag="feat")

        # ---- Phase B: pairwise L1 + exp + batch-sum ----
        for g in range(32):            # octet of batch rows: i = 8g + 2b + h
            pl1 = psum_b.tile([128, 512], F32)
            diffs = []
            for b in range(4):
                dts = [
                    diff_pool.tile([CHUNKS[t][1], 512], BF16, tag=f"d{t}",
                                   name=f"d{t}")
                    for t in range(NCH)
                ]
                for h in range(2):
                    i = 8 * g + 2 * b + h
                    nc.scalar.activation(
                        dts[0][:, 256 * h:256 * (h + 1)],
                        act_bf[0][:],
                        mybir.ActivationFunctionType.Relu,
                        bias=act_neg0[:, i:i + 1],
                        scale=1.0,
                    )
                    for t in range(1, NCH):
                        nc.vector.tensor_scalar(
                            dts[t][:, 256 * h:256 * (h + 1)],
                            act_bf[t][:],
                            act_f32[t][:, i:i + 1],
                            0.0,
                            op0=mybir.AluOpType.subtract,
                            op1=mybir.AluOpType.max,
                        )
                diffs.append(dts)
            # d-reduction on PE: l1[32b + r, 256h + j]; interleave col-groups
            # so the 4 concurrent matmuls overlap on distinct array columns.
            for t in range(NCH):
                for b in range(4):
                    nc.tensor.matmul(
                        pl1[32 * b:32 * b + 32, :],
                        s_tile[0:CHUNKS[t][1], 32 * t:32 * t + 32],
                        diffs[b][t][:],
                        start=(t == 0),
                        stop=False,
                        tile_position=(0, 32 * b),
                    )
            # l1_part = 2*sum(relu) - A[r, j]
            for b in range(4):
                for h in range(2):
                    nc.tensor.matmul(
                        pl1[32 * b:32 * b + 13, 256 * h:256 * (h + 1)],
                        s_tile[0:13, 384:397],
                        a_bf[:],
                        start=False,
                        stop=(h == 1),
                        tile_position=(0, 32 * b),
                    )
            for h in range(2):
                jt = junk_pool.tile([128, 256], BF16, tag="junk")
                nc.scalar.activation(
                    jt[:],
                    pl1[:, 256 * h:256 * (h + 1)],
                    mybir.ActivationFunctionType.Exp,
                    scale=-1.0,
                    accum_out=feat_tile[:, 2 * g + h:2 * g + h + 2 - 1],
                )

        nc.sync.dma_start(out=feat_d[:], in_=feat_tile[:])
    nc.finalize()
    return nc


def _build_s_pack():
    s = np.zeros((128, 416), np.float32)
    q = np.arange(COLS)
    t = q // 128
    p = q % 128
    r = q // DK
    s[p, 32 * t + r] = 2.0          # Sx2: 2*sum(relu)
    s[p, 192 + 32 * t + r] = 1.0    # S1: row-sum table A
    for r in range(NK):
        s[r, 384 + r] = -1.0        # -I13: subtract A[r, j]
    return s.astype(ml_dtypes.bfloat16)


_NC_CACHE = None


def _get_nc():
    global _NC_CACHE
    if _NC_CACHE is None:
        _NC_CACHE = build_nc()
    return _NC_CACHE


def make_in_maps(x, weight):
    x = np.asarray(x, np.float32)
    weight = np.asarray(weight, np.float32)
    xT = np.ascontiguousarray(x.T).astype(ml_dtypes.bfloat16)
    wp = np.zeros((IN_D, COLS * N_CORES), np.float32)
    wp[:, :weight.shape[1]] = weight
    s_pack = _build_s_pack()
    return [
        {
            "xT": xT,
            "w": np.ascontiguousarray(wp[:, COLS * c:COLS * (c + 1)]).astype(
                ml_dtypes.bfloat16),
            "s": s_pack,
        }
        for c in range(N_CORES)
    ]


def assemble(x, results):
    """results: list of per-core dicts with 'feat' [128, 64] fp32."""
    x = np.asarray(x, np.float32)
    feats = []
    for c in range(N_CORES):
        f = np.asarray(results[c]["feat"], np.float32)
        a32 = np.asarray(results[c]["aout"], np.float32)   # [13, 256]
        # f[32b + r, 2g + h] = sum_j exp(-(2 relu_sum - A[r, j])) for
        # i = 8g+2b+h; true features need the exp(-A[r, i]) factor back.
        F = f.reshape(4, 32, 32, 2)[:, :NK]        # [b, r, g, h]
        raw = F.transpose(2, 0, 3, 1).reshape(B, NK)
        feats.append(raw * np.exp(-a32).T)
    features = np.concatenate(feats, axis=1)[:, :100]
    return np.concatenate([x, features], axis=1)


def kernel(x, weight):
    in_maps = make_in_maps(x, weight)
    nc = _get_nc()
    res = run_bass_kernel_spmd(nc, in_maps, list(range(N_CORES)))
    return assemble(x, res.results)
